# revision 14
# baseline (speedup 1.0000x reference)
"""LocalActivationUnit (DIN attention MLP) Trainium2 Bass kernel.

Math: att[b,t] = relu(concat([q,k,q-k,q*k]) @ W0 + b0) @ W1 + b1
Algebraic reduction (W0 = [W0a; W0b; W0c; W0d] in 128-row chunks):
  x @ W0 = q@(W0a+W0c) + k@(W0b-W0c) + (q*k)@W0d
         = k @ M_b + c_b          (per batch b)
  where M_b = Bm + diag(q_b) @ D          [E, H]   (Bm = W0b-W0c, D = W0d)
        c_b = q_b @ A + b0                [H]      (A = W0a+W0c)
So per (b,t) work is ONE contraction of length E=128 instead of 4E=512.

Per-core layout (8 cores, 128 batches each, pure data parallel):
  - PE transposes keys tiles [t,E] -> kT [E,t] via identity matmul
  - mm1: hT[64,200] = M_b.T @ kT_b   (single N=200 fp32 matmul per batch)
  - relu+bias fused on ACT with per-partition bias c_b
  - mm2: att[1,200] = W1.T @ hT, one PSUM partition per batch -> batched
    [64,200] contiguous output DMAs
"""

import sys

sys.path.insert(0, "/opt/trn_rl_repo")

import numpy as np

import concourse.bass as bass
import concourse.tile as tile
from concourse import bacc, masks, mybir
from concourse.bass_utils import run_bass_kernel_spmd

B, T, E, H = 1024, 200, 128, 64
NCORES = 8
BPC = B // NCORES  # 128 batches per core
GRP = 64  # batches per output group (one PSUM partition each)
T0 = 128  # first keys tile rows
T1 = T - T0  # second keys tile rows (72)

F32 = mybir.dt.float32
Alu = mybir.AluOpType
Act = mybir.ActivationFunctionType


def build_nc():
    nc = bacc.Bacc()
    q_d = nc.dram_tensor("query", [BPC, E], F32, kind="ExternalInput")
    keys_d = nc.dram_tensor("keys", [BPC, T, E], F32, kind="ExternalInput")
    w0_d = nc.dram_tensor("W0", [4 * E, H], F32, kind="ExternalInput")
    b0_d = nc.dram_tensor("b0", [H, 1], F32, kind="ExternalInput")
    w1_d = nc.dram_tensor("W1", [H, 1], F32, kind="ExternalInput")
    b1_d = nc.dram_tensor("b1", [1, 1], F32, kind="ExternalInput")
    out_d = nc.dram_tensor("out", [BPC, T], F32, kind="ExternalOutput")

    with tile.TileContext(nc) as tc:
        with (
            tc.tile_pool(name="consts", bufs=1) as consts,
            tc.tile_pool(name="kin", bufs=6) as kin,
            tc.tile_pool(name="mid", bufs=4) as mid,
            tc.tile_pool(name="outp", bufs=2) as outp,
            tc.tile_pool(name="ps_t", bufs=2, space="PSUM") as ps_t,
            tc.tile_pool(name="ps_h", bufs=2, space="PSUM") as ps_h,
            tc.tile_pool(name="ps_o", bufs=2, space="PSUM") as ps_o,
            tc.tile_pool(name="ps_up", bufs=1, space="PSUM") as ps_up,
        ):
            # ---------- constants / setup ----------
            ident = consts.tile([128, 128], F32)
            masks.make_identity(nc, ident[:])

            # W0 chunks side by side: w0_sb[:, c*H:(c+1)*H] = W0[c*128:(c+1)*128, :]
            w0c = []
            for c in range(4):
                t = consts.tile([E, H], F32, tag=f"w0c{c}")
                nc.sync.dma_start(out=t[:], in_=w0_d[c * E : (c + 1) * E, :])
                w0c.append(t)
            a_sb = consts.tile([E, H], F32)  # A = W0a + W0c
            nc.vector.tensor_add(a_sb[:], w0c[0][:], w0c[2][:])
            bm_sb = consts.tile([E, H], F32)  # Bm = W0b - W0c
            nc.vector.tensor_sub(bm_sb[:], w0c[1][:], w0c[2][:])
            d_view = w0c[3][:]  # D = W0d

            b0_sb = consts.tile([H, 1], F32)
            nc.sync.dma_start(out=b0_sb[:], in_=b0_d[:, :])
            w1_sb = consts.tile([H, 1], F32)
            nc.sync.dma_start(out=w1_sb[:], in_=w1_d[:, :])
            b1_sb = consts.tile([1, 1], F32)
            nc.sync.dma_start(out=b1_sb[:], in_=b1_d[:, :])

            # q slab [b, E] -> qT [E, b]
            q_sb = consts.tile([BPC, E], F32)
            nc.sync.dma_start(out=q_sb[:], in_=q_d[:, :])
            qt_ps = ps_up.tile([128, 128], F32, tag="setup")
            nc.tensor.transpose(qt_ps[:], q_sb[:], ident[:])
            qt_sb = consts.tile([E, BPC], F32)
            nc.scalar.copy(qt_sb[:], qt_ps[:])

            # cT[h, b] = (q @ A).T + b0
            ct_ps = ps_up.tile([H, BPC], F32, tag="setup")
            nc.tensor.matmul(ct_ps[:], a_sb[:], qt_sb[:], start=True, stop=True)
            ct_sb = consts.tile([H, BPC], F32)
            nc.scalar.activation(ct_sb[:], ct_ps[:], Act.Identity, bias=b0_sb[:])

            # pair-stacked bias: column p = [c_{p}; c_{p+64}]
            NP = BPC // 2
            ct2_sb = consts.tile([128, NP], F32)
            nc.scalar.copy(ct2_sb[0:H, :], ct_sb[:, 0:NP])
            nc.scalar.copy(ct2_sb[H:128, :], ct_sb[:, NP:BPC])

            # block-diagonal W1: [[W1, 0], [0, W1]]
            w1blk = consts.tile([128, 2], F32)
            nc.vector.memset(w1blk[:], 0.0)
            nc.vector.tensor_copy(w1blk[0:H, 0:1], w1_sb[:])
            nc.vector.tensor_copy(w1blk[H:128, 1:2], w1_sb[:])

            # b1 replicated to 2 partitions
            b1r2 = consts.tile([2, 1], F32)
            nc.sync.dma_start(out=b1r2[0:1, :], in_=b1_d[:, :])
            nc.sync.dma_start(out=b1r2[1:2, :], in_=b1_d[:, :])

            # ---------- main loop: pairs (p, p+64) ----------
            for p in range(NP):
                ia, ib = p, p + NP
                kt_ps = ps_t.tile([128, 2 * T], F32)
                kt_sb = mid.tile([128, 2 * T], F32, tag="kT")
                for half, i in ((0, ia), (1, ib)):
                    k0 = kin.tile([128, E], F32, tag=f"k0{half}")
                    nc.sync.dma_start(out=k0[:], in_=keys_d[i, 0:T0, :])
                    k1 = kin.tile([128, E], F32, tag=f"k1{half}")
                    nc.sync.dma_start(out=k1[0:T1, :], in_=keys_d[i, T0:T, :])
                    o = half * T
                    nc.tensor.transpose(kt_ps[:, o : o + T0], k0[:], ident[:])
                    nc.tensor.transpose(
                        kt_ps[:, o + T0 : o + T], k1[0:T1, :], ident[0:T1, 0:T1]
                    )
                # psum->sbuf copies split across engines
                nc.scalar.copy(kt_sb[:, 0:T], kt_ps[:, 0:T])
                nc.vector.tensor_copy(kt_sb[:, T : 2 * T], kt_ps[:, T : 2 * T])

                # M_b = D * q_b + Bm for both halves
                ht_ps = ps_h.tile([128, T], F32)
                for half, i in ((0, ia), (1, ib)):
                    m_sb = mid.tile([E, H], F32, tag=f"M{half}")
                    nc.vector.scalar_tensor_tensor(
                        out=m_sb[:],
                        in0=d_view,
                        scalar=qt_sb[:, i : i + 1],
                        in1=bm_sb[:],
                        op0=Alu.mult,
                        op1=Alu.add,
                    )
                    nc.tensor.matmul(
                        ht_ps[half * H : (half + 1) * H, :],
                        m_sb[:],
                        kt_sb[:, half * T : (half + 1) * T],
                        start=True,
                        stop=True,
                    )
                ht_sb = mid.tile([128, T], F32, tag="hT")
                nc.scalar.activation(
                    ht_sb[:], ht_ps[:], Act.Relu, bias=ct2_sb[:, p : p + 1]
                )

                att2_ps = ps_o.tile([2, T], F32)
                nc.tensor.matmul(att2_ps[:], w1blk[:], ht_sb[:], start=True, stop=True)
                att2_sb = outp.tile([2, T], F32, tag="att2")
                if p % 2 == 0:
                    nc.vector.tensor_scalar_add(att2_sb[:], att2_ps[:], b1r2[:])
                else:
                    nc.scalar.activation(
                        att2_sb[:], att2_ps[:], Act.Identity, bias=b1r2[:]
                    )
                nc.sync.dma_start(out=out_d[ia : ia + 1, :], in_=att2_sb[0:1, :])
                nc.sync.dma_start(out=out_d[ib : ib + 1, :], in_=att2_sb[1:2, :])
    return nc


_CACHE = {}


def _get_nc():
    if "nc" not in _CACHE:
        nc = build_nc()
        if not nc.is_finalized():
            nc.finalize()
        _CACHE["nc"] = nc
    return _CACHE["nc"]


def kernel(query, keys, W0, b0, W1, b1, _trace=False, **trace_kwargs):
    nc = _get_nc()
    q2 = np.ascontiguousarray(np.asarray(query, np.float32).reshape(B, E))
    k3 = np.ascontiguousarray(np.asarray(keys, np.float32).reshape(B, T, E))
    w0 = np.ascontiguousarray(np.asarray(W0, np.float32))
    b0a = np.ascontiguousarray(np.asarray(b0, np.float32).reshape(H, 1))
    w1 = np.ascontiguousarray(np.asarray(W1, np.float32).reshape(H, 1))
    b1a = np.ascontiguousarray(np.asarray(b1, np.float32).reshape(1, 1))
    in_maps = []
    for c in range(NCORES):
        s = slice(c * BPC, (c + 1) * BPC)
        in_maps.append(
            {
                "query": q2[s],
                "keys": k3[s],
                "W0": w0,
                "b0": b0a,
                "W1": w1,
                "b1": b1a,
            }
        )
    res = run_bass_kernel_spmd(
        nc, in_maps, list(range(NCORES)), trace=_trace, **trace_kwargs
    )
    out = np.concatenate([res.results[c]["out"] for c in range(NCORES)], axis=0)
    out = out.reshape(B, T, 1)
    if _trace:
        return out, res
    return out
